# revision 2
# baseline (speedup 1.0000x reference)
"""DocumentRetriever Trainium2 kernel.

Math (per batch element b, one per NeuronCore, 8 cores):
    q      = query[b] @ W + bias                    [512, 1024]
    scores = (q @ docs[b].T) / sqrt(1024)           [512, 4096]
    scores = where(mask, scores, -inf)
    attn   = softmax(scores, axis=-1)               [512, 4096]  (output)
    retr   = attn @ docs[b]                         [512, 1024]  (output)

Implementation notes:
  - All matmuls run in float32r (TF32-class, ~1e-3 rel err, full PE rate).
  - The PE contracts over the partition dim, so operands of the scores GEMM
    need H on partitions: queryT / docsT are prepared host-side as part of
    sharding (layout prep), along with the mask bias row (replicated) and
    the b/32 bias columns.  softmax skips the max-subtraction (scores are
    ~N(0,1); exp sums are small and fp32-safe) which matches softmax exactly
    up to fp32 rounding.
  - attn is computed in [q-part, d-free] layout (softmax-friendly); the
    weighted-sum GEMM needs attn with d on partitions, produced on-chip with
    PE transpose-mode; normalization by 1/sum is folded into the final
    evictions of both outputs.
"""
import numpy as np
import time

import jax
import jax.numpy as jnp
from jax.sharding import Mesh, PartitionSpec, NamedSharding
from jax.experimental.shard_map import shard_map

import concourse.bacc as bacc
import concourse.tile as tile
from concourse import mybir
from concourse.bass2jax import _bass_exec_p, install_neuronx_cc_hook

B, Q, D, H = 8, 512, 4096, 1024
N_CORES = 8
QT, OT = Q // 128, H // 128          # 4 q-tiles, 8 h-tiles
NCH = D // 512                       # 8 d-chunks of 512
F32 = mybir.dt.float32
F32R = mybir.dt.float32r
MASK_NEG = np.float32(-1e9)


def build_nc(reps: int = 1):
    nc = bacc.Bacc("TRN2", target_bir_lowering=False, debug=False,
                   num_devices=N_CORES)
    queryT_ap = nc.dram_tensor("queryT", [H, Q], F32R, kind="ExternalInput").ap()
    wmat_ap = nc.dram_tensor("wmat", [H, H], F32R, kind="ExternalInput").ap()
    bias32_ap = nc.dram_tensor("bias32", [128, OT], F32, kind="ExternalInput").ap()
    docsT_ap = nc.dram_tensor("docsT", [H, D], F32R, kind="ExternalInput").ap()
    docs_ap = nc.dram_tensor("docs", [D, H], F32R, kind="ExternalInput").ap()
    mbias_ap = nc.dram_tensor("mbias", [128, D], F32, kind="ExternalInput").ap()
    ident_ap = nc.dram_tensor("ident", [128, 128], F32R, kind="ExternalInput").ap()
    attn_ap = nc.dram_tensor("attn_out", [Q, D], F32, kind="ExternalOutput").ap()
    retr_ap = nc.dram_tensor("retr_out", [Q, H], F32, kind="ExternalOutput").ap()

    Ident = mybir.ActivationFunctionType.Identity
    Exp = mybir.ActivationFunctionType.Exp
    Copy = mybir.ActivationFunctionType.Copy
    AX = mybir.AxisListType.X

    with tile.TileContext(nc) as tc:
        with (
            tc.tile_pool(name="const", bufs=1) as constp,
            tc.tile_pool(name="qT", bufs=1) as qTp,
            tc.tile_pool(name="expS", bufs=1) as expp,
            tc.tile_pool(name="acc", bufs=1) as accp,
            tc.tile_pool(name="small", bufs=1) as smallp,
            tc.tile_pool(name="qtt", bufs=8) as qttp,
            tc.tile_pool(name="w", bufs=4) as wp,
            tc.tile_pool(name="dT", bufs=12) as dTp,
            tc.tile_pool(name="dn", bufs=6) as dnp,
            tc.tile_pool(name="aT", bufs=6) as aTp,
            tc.tile_pool(name="stagA", bufs=3) as stagAp,
            tc.tile_pool(name="stagR", bufs=2) as stagRp,
            tc.tile_pool(name="psP", bufs=2, space="PSUM") as psP,
            tc.tile_pool(name="psS", bufs=2, space="PSUM") as psS,
            tc.tile_pool(name="psT", bufs=2, space="PSUM") as psT,
            tc.tile_pool(name="psC", bufs=2, space="PSUM") as psC,
        ):
            def body():
                # ---- constants ----
                mb = constp.tile([128, D], F32, tag="mb")
                nc.sync.dma_start(mb[:], mbias_ap[:])
                idt = constp.tile([128, 128], F32R, tag="idt")
                nc.sync.dma_start(idt[:], ident_ap[:])
                b32 = constp.tile([128, OT], F32, tag="b32")
                nc.sync.dma_start(b32[:], bias32_ap[:])

                # ---- persistent intermediates ----
                qT = [qTp.tile([128, Q], F32R, tag=f"qT{o}", name=f"qT{o}") for o in range(OT)]
                expS = [expp.tile([128, D], F32R, tag=f"expS{qt}", name=f"expS{qt}") for qt in range(QT)]
                partials = [smallp.tile([128, NCH], F32, tag=f"pa{qt}", name=f"pa{qt}") for qt in range(QT)]
                recips = [smallp.tile([128, 1], F32, tag=f"re{qt}", name=f"re{qt}") for qt in range(QT)]
                c_acc = [[accp.tile([128, 512], F32, tag=f"ca{qt}_{ht}", name=f"ca{qt}_{ht}")
                          for ht in range(2)] for qt in range(QT)]

                # ---- projection: qT[o] = (W.T-block @ queryT + b) / 32 ----
                qtt = []
                for i in range(OT):
                    t = qttp.tile([128, Q], F32R)
                    nc.sync.dma_start(t[:], queryT_ap[i * 128:(i + 1) * 128, :])
                    qtt.append(t)
                for o in range(OT):
                    ps = psP.tile([128, Q], F32)
                    for i in range(OT):
                        wt = wp.tile([128, 128], F32R)
                        nc.sync.dma_start(
                            wt[:], wmat_ap[i * 128:(i + 1) * 128,
                                           o * 128:(o + 1) * 128])
                        nc.tensor.matmul(ps[:], wt[:], qtt[i][:],
                                         start=(i == 0), stop=(i == OT - 1))
                    nc.scalar.activation(qT[o][:], ps[:], Ident,
                                         bias=b32[:, o:o + 1], scale=1.0 / 32.0)

                # ---- main loop over d-chunks ----
                for c in range(NCH):
                    dTt = []
                    for o in range(OT):
                        t = dTp.tile([128, 512], F32R)
                        nc.sync.dma_start(
                            t[:], docsT_ap[o * 128:(o + 1) * 128,
                                           c * 512:(c + 1) * 512])
                        dTt.append(t)
                    dnat = []
                    for dt in range(4):
                        t = dnp.tile([128, H], F32R)
                        r0 = c * 512 + dt * 128
                        nc.sync.dma_start(t[:], docs_ap[r0:r0 + 128, :])
                        dnat.append(t)

                    # scores -> +maskbias -> exp (with per-chunk row sums)
                    for qt in range(QT):
                        ps = psS.tile([128, 512], F32)
                        for o in range(OT):
                            nc.tensor.matmul(
                                ps[:], qT[o][:, qt * 128:(qt + 1) * 128],
                                dTt[o][:], start=(o == 0), stop=(o == OT - 1))
                        nc.vector.tensor_add(ps[:], ps[:],
                                             mb[:, c * 512:(c + 1) * 512])
                        nc.scalar.activation(
                            expS[qt][:, c * 512:(c + 1) * 512], ps[:], Exp,
                            accum_out=partials[qt][:, c:c + 1])

                    # transpose exp block-wise -> attnT tiles [128d, 512q]
                    aTt = []
                    for dt in range(4):
                        tps = psT.tile([128, 512], F32R)
                        for qt in range(QT):
                            nc.tensor.transpose(
                                tps[:, qt * 128:(qt + 1) * 128],
                                expS[qt][:, c * 512 + dt * 128:
                                          c * 512 + (dt + 1) * 128],
                                idt[:])
                        st = aTp.tile([128, 512], F32R)
                        nc.scalar.copy(st[:], tps[:])
                        aTt.append(st)

                    # weighted sum: accumulate attnT.T @ docs into c_acc
                    for qt in range(QT):
                        for ht in range(2):
                            ps = psC.tile([128, 512], F32)
                            for dt in range(4):
                                nc.tensor.matmul(
                                    ps[:], aTt[dt][:, qt * 128:(qt + 1) * 128],
                                    dnat[dt][:, ht * 512:(ht + 1) * 512],
                                    start=(dt == 0), stop=(dt == 3))
                            if c == 0:
                                nc.vector.tensor_copy(c_acc[qt][ht][:], ps[:])
                            else:
                                nc.vector.tensor_add(c_acc[qt][ht][:],
                                                     c_acc[qt][ht][:], ps[:])

                # ---- finalize ----
                for qt in range(QT):
                    tot = smallp.tile([128, 1], F32, tag=f"tot{qt}", name=f"tot{qt}")
                    nc.vector.reduce_sum(tot[:], partials[qt][:], axis=AX)
                    nc.vector.reciprocal(recips[qt][:], tot[:])

                for qt in range(QT):
                    for c in range(NCH):
                        sa = stagAp.tile([128, 512], F32)
                        nc.scalar.activation(
                            sa[:], expS[qt][:, c * 512:(c + 1) * 512], Copy,
                            scale=recips[qt][:])
                        nc.sync.dma_start(
                            attn_ap[qt * 128:(qt + 1) * 128,
                                    c * 512:(c + 1) * 512], sa[:])
                for qt in range(QT):
                    for ht in range(2):
                        sr = stagRp.tile([128, 512], F32)
                        nc.vector.tensor_scalar_mul(sr[:], c_acc[qt][ht][:],
                                                    recips[qt][:])
                        nc.sync.dma_start(
                            retr_ap[qt * 128:(qt + 1) * 128,
                                    ht * 512:(ht + 1) * 512], sr[:])

            if reps == 1:
                body()
            else:
                with tc.For_i(0, reps, 1):
                    body()
    nc.compile()
    return nc


# ---------------- PJRT SPMD runner ----------------

class SpmdRunner:
    def __init__(self, nc, n_cores=N_CORES):
        install_neuronx_cc_hook()
        self.nc = nc
        self.n_cores = n_cores
        partition_name = nc.partition_id_tensor.name if nc.partition_id_tensor else None
        in_names, out_names, out_avals = [], [], []
        for alloc in nc.m.functions[0].allocations:
            if not isinstance(alloc, mybir.MemoryLocationSet):
                continue
            name = alloc.memorylocations[0].name
            if alloc.kind == "ExternalInput":
                if name != partition_name:
                    in_names.append(name)
            elif alloc.kind == "ExternalOutput":
                out_names.append(name)
                out_avals.append(jax.core.ShapedArray(
                    tuple(alloc.tensor_shape), mybir.dt.np(alloc.dtype)))
        self.partition_name = partition_name
        self.in_names, self.out_names, self.out_avals = in_names, out_names, out_avals
        n_params = len(in_names)
        self.n_params = n_params
        all_in_names = list(in_names) + list(out_names)
        if partition_name is not None:
            all_in_names.append(partition_name)

        def _body(*args):
            outs = _bass_exec_p.bind(
                *args,
                out_avals=tuple(out_avals),
                in_names=tuple(all_in_names),
                out_names=tuple(out_names),
                lowering_input_output_aliases=(),
                sim_require_finite=True,
                sim_require_nnan=True,
                nc=nc,
            )
            return tuple(outs)

        devices = jax.devices()[:n_cores]
        self.mesh = Mesh(np.asarray(devices), ("core",))
        n_outs = len(out_names)
        n_extra = 1 if partition_name is not None else 0
        in_specs = (PartitionSpec("core"),) * (n_params + n_outs + n_extra)
        out_specs = (PartitionSpec("core"),) * n_outs
        donate = tuple(range(n_params, n_params + n_outs))
        self._fn = jax.jit(
            shard_map(_body, mesh=self.mesh, in_specs=in_specs,
                      out_specs=out_specs, check_rep=False),
            donate_argnums=donate, keep_unused=True)
        self._sharding = NamedSharding(self.mesh, PartitionSpec("core"))
        zero_shapes = [((n_cores * av.shape[0], *av.shape[1:]), av.dtype)
                       for av in out_avals]
        sh = self._sharding
        self._make_zeros = jax.jit(
            lambda: tuple(jnp.zeros(s, d) for s, d in zero_shapes),
            out_shardings=tuple(sh for _ in zero_shapes))
        self._dev_in = None
        self._dev_pid = None

    def stage_inputs(self, in_maps):
        args = []
        for name in self.in_names:
            concat = np.concatenate([np.asarray(m[name]) for m in in_maps], axis=0)
            args.append(jax.device_put(concat, self._sharding))
        self._dev_in = args
        if self.partition_name is not None:
            pid = np.arange(self.n_cores, dtype=np.uint32).reshape(self.n_cores, 1)
            self._dev_pid = jax.device_put(pid, self._sharding)

    def run(self):
        zeros = list(self._make_zeros())
        args = list(self._dev_in) + zeros
        if self._dev_pid is not None:
            args.append(self._dev_pid)
        outs = self._fn(*args)
        jax.block_until_ready(outs)
        return outs

    def results(self, outs):
        res = []
        for c in range(self.n_cores):
            d = {}
            for i, name in enumerate(self.out_names):
                per = np.asarray(outs[i]).reshape(
                    self.n_cores, *self.out_avals[i].shape)
                d[name] = per[c]
            res.append(d)
        return res


_RUNNERS = {}


def _get_runner(reps=1):
    if reps not in _RUNNERS:
        nc = build_nc(reps)
        _RUNNERS[reps] = SpmdRunner(nc, N_CORES)
    return _RUNNERS[reps]


def make_in_maps(query, docs, mask, w, b):
    bias32 = np.ascontiguousarray(
        (b.astype(np.float32) / 32.0).reshape(OT, 128).T)
    ident = np.eye(128, dtype=np.float32)
    w32 = np.ascontiguousarray(w.astype(np.float32))
    in_maps = []
    for core in range(N_CORES):
        mb_row = np.where(mask[core] == 0, MASK_NEG, np.float32(0.0)).astype(np.float32)
        in_maps.append({
            "queryT": np.ascontiguousarray(query[core].T),
            "wmat": w32,
            "bias32": bias32,
            "docsT": np.ascontiguousarray(docs[core].T),
            "docs": np.ascontiguousarray(docs[core]),
            "mbias": np.ascontiguousarray(
                np.broadcast_to(mb_row[None, :], (128, D))),
            "ident": ident,
        })
    return in_maps


def kernel(**inputs):
    query = np.asarray(inputs["query_embeddings"], dtype=np.float32)
    docs = np.asarray(inputs["document_embeddings"], dtype=np.float32)
    mask = np.asarray(inputs["attention_mask"])
    w = np.asarray(inputs["proj_w"], dtype=np.float32)
    b = np.asarray(inputs["proj_b"], dtype=np.float32)

    runner = _get_runner(reps=1)
    runner.stage_inputs(make_in_maps(query, docs, mask, w, b))
    outs = runner.run()
    res = runner.results(outs)
    retrieved = np.stack([res[c]["retr_out"] for c in range(N_CORES)])
    attn = np.stack([res[c]["attn_out"] for c in range(N_CORES)])
    return retrieved, attn


# revision 6
# speedup vs baseline: 1.5196x; 1.5196x over previous
"""DocumentRetriever Trainium2 kernel.

Math (per batch element b, one per NeuronCore, 8 cores):
    q      = query[b] @ W + bias                    [512, 1024]
    scores = (q @ docs[b].T) / sqrt(1024)           [512, 4096]
    scores = where(mask, scores, -inf)
    attn   = softmax(scores, axis=-1)               [512, 4096]  (output)
    retr   = attn @ docs[b]                         [512, 1024]  (output)

Implementation notes:
  - All matmuls run in float32r (TF32-class, ~1e-3 rel err, full PE rate).
  - The PE contracts over the partition dim, so operands of the scores GEMM
    need H on partitions: queryT / docsT are prepared host-side as part of
    sharding (layout prep), along with the mask bias row (replicated) and
    the b/32 bias columns.  softmax skips the max-subtraction (scores are
    ~N(0,1); exp sums are small and fp32-safe) which matches softmax exactly
    up to fp32 rounding.
  - attn is computed in [q-part, d-free] layout (softmax-friendly); the
    weighted-sum GEMM needs attn with d on partitions, produced on-chip with
    PE transpose-mode; normalization by 1/sum is folded into the final
    evictions of both outputs.
"""
import numpy as np
import time

import jax
import jax.numpy as jnp
from jax.sharding import Mesh, PartitionSpec, NamedSharding
from jax.experimental.shard_map import shard_map

import concourse.bacc as bacc
import concourse.tile as tile
from concourse import mybir
from concourse.bass2jax import _bass_exec_p, install_neuronx_cc_hook

B, Q, D, H = 8, 512, 4096, 1024
N_CORES = 8
QT, OT = Q // 128, H // 128          # 4 q-tiles, 8 h-tiles
NCH = D // 512                       # 8 d-chunks of 512
F32 = mybir.dt.float32
F32R = mybir.dt.float32r
F16 = mybir.dt.float16
MASK_NEG = np.float32(-1e9)


def build_nc(reps: int = 1, variant: str = "full"):
    nc = bacc.Bacc("TRN2", target_bir_lowering=False, debug=False,
                   num_devices=N_CORES)
    queryT_ap = nc.dram_tensor("queryT", [H, Q], F16, kind="ExternalInput").ap()
    wmat_ap = nc.dram_tensor("wmat", [H, H], F16, kind="ExternalInput").ap()
    bias32_ap = nc.dram_tensor("bias32", [128, OT], F32, kind="ExternalInput").ap()
    docsT_ap = nc.dram_tensor("docsT", [H, D], F16, kind="ExternalInput").ap()
    docs_ap = nc.dram_tensor("docs", [D, H], F16, kind="ExternalInput").ap()
    mbias_ap = nc.dram_tensor("mbias", [128, D], F32, kind="ExternalInput").ap()
    ident_ap = nc.dram_tensor("ident", [128, 128], F16, kind="ExternalInput").ap()
    attn_ap = nc.dram_tensor("attn_out", [Q, D], F32, kind="ExternalOutput").ap()
    retr_ap = nc.dram_tensor("retr_out", [Q, H], F32, kind="ExternalOutput").ap()

    Ident = mybir.ActivationFunctionType.Identity
    Exp = mybir.ActivationFunctionType.Exp
    Copy = mybir.ActivationFunctionType.Copy
    AX = mybir.AxisListType.X

    do_dma = variant != "computeonly"
    do_compute = variant != "dmaonly"
    ilv = variant == "ilv"

    with tile.TileContext(nc) as tc:
        with (
            tc.tile_pool(name="const", bufs=1) as constp,
            tc.tile_pool(name="qT", bufs=1) as qTp,
            tc.tile_pool(name="expS", bufs=1) as expp,
            tc.tile_pool(name="acc", bufs=1) as accp,
            tc.tile_pool(name="small", bufs=1) as smallp,
            tc.tile_pool(name="qtt", bufs=8) as qttp,
            tc.tile_pool(name="w", bufs=4) as wp,
            tc.tile_pool(name="dT", bufs=12) as dTp,
            tc.tile_pool(name="dn", bufs=6) as dnp,
            tc.tile_pool(name="aT", bufs=6) as aTp,
            tc.tile_pool(name="stagA", bufs=3) as stagAp,
            tc.tile_pool(name="stagR", bufs=2) as stagRp,
            tc.tile_pool(name="psP", bufs=2, space="PSUM") as psP,
            tc.tile_pool(name="psS", bufs=2, space="PSUM") as psS,
            tc.tile_pool(name="psT", bufs=2, space="PSUM") as psT,
            tc.tile_pool(name="psC", bufs=2, space="PSUM") as psC,
        ):
            def body():
                # ---- constants ----
                mb = constp.tile([128, D], F32, tag="mb")
                nc.sync.dma_start(mb[:], mbias_ap[:])
                idt = constp.tile([128, 128], F16, tag="idt")
                nc.sync.dma_start(idt[:], ident_ap[:])
                b32 = constp.tile([128, OT], F32, tag="b32")
                nc.sync.dma_start(b32[:], bias32_ap[:])

                # ---- persistent intermediates ----
                qT = [qTp.tile([128, Q], F16, tag=f"qT{o}", name=f"qT{o}") for o in range(OT)]
                expS = [expp.tile([128, D], F16, tag=f"expS{qt}", name=f"expS{qt}") for qt in range(QT)]
                partials = [smallp.tile([128, NCH], F32, tag=f"pa{qt}", name=f"pa{qt}") for qt in range(QT)]
                recips = [smallp.tile([128, 1], F32, tag=f"re{qt}", name=f"re{qt}") for qt in range(QT)]
                c_acc = [[accp.tile([128, 512], F32, tag=f"ca{qt}_{ht}", name=f"ca{qt}_{ht}")
                          for ht in range(2)] for qt in range(QT)]

                # ---- projection: qT[o] = (W.T-block @ queryT + b) / 32 ----
                qtt = []
                for i in range(OT):
                    t = qttp.tile([128, Q], F16)
                    if do_dma:
                        nc.sync.dma_start(t[:], queryT_ap[i * 128:(i + 1) * 128, :])
                    qtt.append(t)
                for o in range(OT):
                    ps = psP.tile([128, Q], F32)
                    for i in range(OT):
                        wt = wp.tile([128, 128], F16)
                        if do_dma:
                            nc.sync.dma_start(
                                wt[:], wmat_ap[i * 128:(i + 1) * 128,
                                               o * 128:(o + 1) * 128])
                        if do_compute:
                            nc.tensor.matmul(ps[:], wt[:], qtt[i][:],
                                             start=(i == 0), stop=(i == OT - 1))
                    if do_compute:
                        nc.scalar.activation(qT[o][:], ps[:], Ident,
                                             bias=b32[:, o:o + 1], scale=1.0 / 32.0)

                # ---- main loop over d-chunks ----
                for c in range(NCH):
                    dTt = []
                    for o in range(OT):
                        t = dTp.tile([128, 512], F16)
                        if do_dma:
                            nc.sync.dma_start(
                                t[:], docsT_ap[o * 128:(o + 1) * 128,
                                               c * 512:(c + 1) * 512])
                        dTt.append(t)
                    dnat = []
                    for dt in range(4):
                        t = dnp.tile([128, H], F16)
                        r0 = c * 512 + dt * 128
                        if do_dma:
                            nc.sync.dma_start(t[:], docs_ap[r0:r0 + 128, :])
                        dnat.append(t)

                    # scores -> +maskbias -> exp (with per-chunk row sums)
                    if do_compute:
                        for qt in range(QT):
                            ps = psS.tile([128, 512], F32)
                            for o in range(OT):
                                nc.tensor.matmul(
                                    ps[:], qT[o][:, qt * 128:(qt + 1) * 128],
                                    dTt[o][:], start=(o == 0), stop=(o == OT - 1))
                            nc.vector.tensor_add(ps[:], ps[:],
                                                 mb[:, c * 512:(c + 1) * 512])
                            nc.scalar.activation(
                                expS[qt][:, c * 512:(c + 1) * 512], ps[:], Exp,
                                accum_out=partials[qt][:, c:c + 1])

                    # transpose exp block-wise -> attnT tiles [128d, 512q]
                    if do_compute:
                        aTt = [None] * 4

                        def emit_transpose(dt):
                            tps = psT.tile([128, 512], F16, name=f"tps{c}_{dt}", tag="tps")
                            for qt in range(QT):
                                nc.tensor.transpose(
                                    tps[:, qt * 128:(qt + 1) * 128],
                                    expS[qt][:, c * 512 + dt * 128:
                                              c * 512 + (dt + 1) * 128],
                                    idt[:])
                            st = aTp.tile([128, 512], F16, name=f"aT{c}_{dt}", tag="aT")
                            nc.scalar.copy(st[:], tps[:])
                            aTt[dt] = st

                        if not ilv:
                            for dt in range(4):
                                emit_transpose(dt)

                        # weighted sum: accumulate attnT.T @ docs into c_acc
                        for qt in range(QT):
                            for ht in range(2):
                                ps = psC.tile([128, 512], F32)
                                for dt in range(4):
                                    if ilv and aTt[dt] is None:
                                        emit_transpose(dt)
                                    nc.tensor.matmul(
                                        ps[:], aTt[dt][:, qt * 128:(qt + 1) * 128],
                                        dnat[dt][:, ht * 512:(ht + 1) * 512],
                                        start=(dt == 0), stop=(dt == 3))
                                if c == 0:
                                    nc.vector.tensor_copy(c_acc[qt][ht][:], ps[:])
                                else:
                                    nc.vector.tensor_add(c_acc[qt][ht][:],
                                                         c_acc[qt][ht][:], ps[:])

                # ---- finalize ----
                if not do_compute:
                    # dmaonly: write mb-sourced bytes to outputs (same volume)
                    for qt in range(QT):
                        for c in range(NCH):
                            nc.sync.dma_start(
                                attn_ap[qt * 128:(qt + 1) * 128,
                                        c * 512:(c + 1) * 512],
                                mb[:, c * 512:(c + 1) * 512])
                    for qt in range(QT):
                        for ht in range(2):
                            nc.sync.dma_start(
                                retr_ap[qt * 128:(qt + 1) * 128,
                                        ht * 512:(ht + 1) * 512],
                                mb[:, ht * 512:(ht + 1) * 512])
                    return
                for qt in range(QT):
                    tot = smallp.tile([128, 1], F32, tag=f"tot{qt}", name=f"tot{qt}")
                    nc.vector.reduce_sum(tot[:], partials[qt][:], axis=AX)
                    nc.vector.reciprocal(recips[qt][:], tot[:])

                for qt in range(QT):
                    for c in range(NCH):
                        sa = stagAp.tile([128, 512], F32)
                        nc.scalar.activation(
                            sa[:], expS[qt][:, c * 512:(c + 1) * 512], Copy,
                            scale=recips[qt][:])
                        nc.sync.dma_start(
                            attn_ap[qt * 128:(qt + 1) * 128,
                                    c * 512:(c + 1) * 512], sa[:])
                for qt in range(QT):
                    for ht in range(2):
                        sr = stagRp.tile([128, 512], F32)
                        nc.vector.tensor_scalar_mul(sr[:], c_acc[qt][ht][:],
                                                    recips[qt][:])
                        nc.sync.dma_start(
                            retr_ap[qt * 128:(qt + 1) * 128,
                                    ht * 512:(ht + 1) * 512], sr[:])

            if reps == 1:
                body()
            else:
                with tc.For_i(0, reps, 1):
                    body()
    nc.compile()
    return nc


# ---------------- PJRT SPMD runner ----------------

class SpmdRunner:
    def __init__(self, nc, n_cores=N_CORES):
        install_neuronx_cc_hook()
        self.nc = nc
        self.n_cores = n_cores
        partition_name = nc.partition_id_tensor.name if nc.partition_id_tensor else None
        in_names, out_names, out_avals = [], [], []
        for alloc in nc.m.functions[0].allocations:
            if not isinstance(alloc, mybir.MemoryLocationSet):
                continue
            name = alloc.memorylocations[0].name
            if alloc.kind == "ExternalInput":
                if name != partition_name:
                    in_names.append(name)
            elif alloc.kind == "ExternalOutput":
                out_names.append(name)
                out_avals.append(jax.core.ShapedArray(
                    tuple(alloc.tensor_shape), mybir.dt.np(alloc.dtype)))
        self.partition_name = partition_name
        self.in_names, self.out_names, self.out_avals = in_names, out_names, out_avals
        n_params = len(in_names)
        self.n_params = n_params
        all_in_names = list(in_names) + list(out_names)
        if partition_name is not None:
            all_in_names.append(partition_name)

        def _body(*args):
            outs = _bass_exec_p.bind(
                *args,
                out_avals=tuple(out_avals),
                in_names=tuple(all_in_names),
                out_names=tuple(out_names),
                lowering_input_output_aliases=(),
                sim_require_finite=True,
                sim_require_nnan=True,
                nc=nc,
            )
            return tuple(outs)

        devices = jax.devices()[:n_cores]
        self.mesh = Mesh(np.asarray(devices), ("core",))
        n_outs = len(out_names)
        n_extra = 1 if partition_name is not None else 0
        in_specs = (PartitionSpec("core"),) * (n_params + n_outs + n_extra)
        out_specs = (PartitionSpec("core"),) * n_outs
        donate = tuple(range(n_params, n_params + n_outs))
        self._fn = jax.jit(
            shard_map(_body, mesh=self.mesh, in_specs=in_specs,
                      out_specs=out_specs, check_rep=False),
            donate_argnums=donate, keep_unused=True)
        self._sharding = NamedSharding(self.mesh, PartitionSpec("core"))
        zero_shapes = [((n_cores * av.shape[0], *av.shape[1:]), av.dtype)
                       for av in out_avals]
        sh = self._sharding
        self._make_zeros = jax.jit(
            lambda: tuple(jnp.zeros(s, d) for s, d in zero_shapes),
            out_shardings=tuple(sh for _ in zero_shapes))
        self._dev_in = None
        self._dev_pid = None

    def stage_inputs(self, in_maps):
        args = []
        for name in self.in_names:
            concat = np.concatenate([np.asarray(m[name]) for m in in_maps], axis=0)
            args.append(jax.device_put(concat, self._sharding))
        self._dev_in = args
        if self.partition_name is not None:
            pid = np.arange(self.n_cores, dtype=np.uint32).reshape(self.n_cores, 1)
            self._dev_pid = jax.device_put(pid, self._sharding)

    def run(self):
        zeros = list(self._make_zeros())
        args = list(self._dev_in) + zeros
        if self._dev_pid is not None:
            args.append(self._dev_pid)
        outs = self._fn(*args)
        jax.block_until_ready(outs)
        return outs

    def results(self, outs):
        res = []
        for c in range(self.n_cores):
            d = {}
            for i, name in enumerate(self.out_names):
                per = np.asarray(outs[i]).reshape(
                    self.n_cores, *self.out_avals[i].shape)
                d[name] = per[c]
            res.append(d)
        return res


_RUNNERS = {}


def _get_runner(reps=1, variant="full"):
    key = (reps, variant)
    if key not in _RUNNERS:
        nc = build_nc(reps, variant)
        _RUNNERS[key] = SpmdRunner(nc, N_CORES)
    return _RUNNERS[key]


def make_in_maps(query, docs, mask, w, b):
    bias32 = np.ascontiguousarray(
        (b.astype(np.float32) / 32.0).reshape(OT, 128).T)
    ident = np.eye(128, dtype=np.float16)
    w16 = np.ascontiguousarray(w.astype(np.float16))
    in_maps = []
    for core in range(N_CORES):
        mb_row = np.where(mask[core] == 0, MASK_NEG, np.float32(0.0)).astype(np.float32)
        in_maps.append({
            "queryT": np.ascontiguousarray(query[core].T).astype(np.float16),
            "wmat": w16,
            "bias32": bias32,
            "docsT": np.ascontiguousarray(docs[core].T).astype(np.float16),
            "docs": docs[core].astype(np.float16),
            "mbias": np.ascontiguousarray(
                np.broadcast_to(mb_row[None, :], (128, D))),
            "ident": ident,
        })
    return in_maps


def kernel(**inputs):
    query = np.asarray(inputs["query_embeddings"], dtype=np.float32)
    docs = np.asarray(inputs["document_embeddings"], dtype=np.float32)
    mask = np.asarray(inputs["attention_mask"])
    w = np.asarray(inputs["proj_w"], dtype=np.float32)
    b = np.asarray(inputs["proj_b"], dtype=np.float32)

    runner = _get_runner(reps=1)
    runner.stage_inputs(make_in_maps(query, docs, mask, w, b))
    outs = runner.run()
    res = runner.results(outs)
    retrieved = np.stack([res[c]["retr_out"] for c in range(N_CORES)])
    attn = np.stack([res[c]["attn_out"] for c in range(N_CORES)])
    return retrieved, attn
